# revision 37
# baseline (speedup 1.0000x reference)
"""Trainium2 Bass kernel for nn_BottleneckS4D (8-core SPMD).

Strategy (self-contained, hardcoded):
  The reference is  u = x_flat @ Wb.T + bb  (256 x 150528 @ 150528 x 1280,
  770MB weight) followed by an S4D block whose output is only consumed at
  the LAST timestep (readout takes y[:, -1, :]), so the FFT convolution
  collapses to a per-channel dot product over time with the reversed S4D
  kernel, and everything downstream is tiny.

  Sharding: split the CONTRACTION dim D_IN=150528 across the 8 cores
  (18816 each). Each core streams its weight slice + x slice once in
  bf16 (48.2MB + 9.6MB, the HBM-traffic minimum at passing precision;
  fp8 fails the 2e-2 gate at ~2.5-3.6e-2) and computes a partial u^T
  (1280, 256) in PSUM with bf16 matmuls at full PE rate. The S4D conv
  is linear in u, so each core reduces its partial u to a partial
  y_last (1280, 4) with the host-precomputed reversed kernel k_rev;
  the cross-core sum uses AllGather + on-core reduction (a mesh
  AllReduce costs ~27us here, AllGather only ~7us). GELU + the GLU
  1x1 conv run on a 160-channel shard per core, a partial readout
  matvec reduces the shard to (64, 4), and a second tiny AllGather +
  reduction gives every core the readout input; core 0's output is
  returned.

  Perf details: weights/x are host-repacked to partition-major layout
  (wTp[p, k, :] = wT[k*128+p, :]) so each DMA chunk moves its k-tiles
  with one large contiguous descriptor per partition; w/x chunks
  alternate between the sync and scalar HWDGE queues (gpsimd only
  carries late-needed smalls — its queue blocks behind collective
  triggers); krev, the D-skip vector and the bias-path term
  bb*(sum_t krev + D)/8 are computed on the host (they depend only on
  weights) so the device does no S4D prep; the first three (small)
  chunk DMAs are issued before anything else and a short PE warmup
  burst lifts the HAM clock gate while they fly; GELU is computed as
  0.5*y*(1+erf(y/sqrt2)) with the 0.5 folded into Wc so every tail
  activation (erf/sigmoid/identity/relu) lives in one act-function set,
  preloaded during the stream; PSUM accumulation restarts at k=KSPLIT
  so the conv of the first half overlaps the remaining stream, and the
  final sub-chunk runs bank-outer so the tail conv pipelines behind the
  PE; the epilogue (GLU + readout) weights are bf16.
"""
import sys

sys.path.insert(0, "/opt/trn_rl_repo")
import numpy as np

B, T, H, N2 = 4, 64, 1280, 32
DIN = 224 * 224 * 3  # 150528
R_HID, NCLS = 64, 60
NCORES = 8
KS = DIN // NCORES   # 18816
KT = KS // 128       # 147
MT = H // 128        # 10
TOK = B * T          # 256
GO = H // NCORES     # 160 GLU output channels per core
HGO = GO // 2        # 80
CH = 7               # max k-tiles per DMA chunk (SBUF slot size)
# sub-chunk schedule: small chunks first so the PE starts fast, then
# 7-k-tile chunks; alternates between the sync and scalar HWDGE queues
SUBS = [(0, 1), (1, 1), (2, 2), (4, 3), (7, 4), (11, 3)] + \
       [(14 + 7 * i, 7) for i in range(19)]
KSPLIT = 98          # conv of k<KSPLIT overlaps the remaining stream
# packedA cols: krev (MT*T) | D4 (MT*B, b-replicated) | biasy (MT*B)
PA_COLS = MT * T + 2 * MT * B  # 720
# packedB cols: wcT(10*320) | w1a(64) | w1b(64) | w2(60)
PB_COLS = MT * 4 * HGO + 2 * R_HID + NCLS  # 3388

_compiled = None


def _build():
    import concourse.bacc as bacc
    import concourse.mybir as mybir
    import concourse.tile as tile
    from concourse.tile import add_dep_helper

    f32 = mybir.dt.float32
    f32r = mybir.dt.float32r
    bf16 = mybir.dt.bfloat16
    AF = mybir.ActivationFunctionType
    OP = mybir.AluOpType
    RG = [list(range(NCORES))]

    nc = bacc.Bacc("TRN2", target_bir_lowering=False, debug=False,
                   num_devices=NCORES)

    d_xT = nc.dram_tensor("xT", [128, KT * TOK], bf16, kind="ExternalInput").ap()
    d_wT = nc.dram_tensor("wT", [128, KT * H], bf16, kind="ExternalInput").ap()
    d_pa = nc.dram_tensor("packedA", [128, PA_COLS], f32,
                          kind="ExternalInput").ap()
    d_pb = nc.dram_tensor("packedB", [128, PB_COLS], bf16,
                          kind="ExternalInput").ap()
    # packedS cols: b1 | b2 | bc(4 cols, 80 rows)
    d_ps = nc.dram_tensor("packedS", [128, 6], f32, kind="ExternalInput").ap()
    d_out = nc.dram_tensor("out", [NCLS, B], f32, kind="ExternalOutput").ap()

    O_D4 = MT * T        # 640
    O_BY = MT * T + MT * B  # 680

    with tile.TileContext(nc) as tc:
        with tc.tile_pool(name="cpool", bufs=1) as cpool, \
             tc.tile_pool(name="dram", bufs=1, space="DRAM") as dp, \
             tc.tile_pool(name="wp", bufs=6) as wp, \
             tc.tile_pool(name="xp", bufs=6) as xp, \
             tc.tile_pool(name="ev", bufs=3) as ev:
            # cross-core exchange buffers: raw [128, MT*B] layout so every
            # scatter/gather DMA keeps >=80B contiguous runs per partition
            ag2_in = dp.tile([128, MT * B], bf16, tag="ag2i")
            ag2_out = dp.tile([NCORES * 128, MT * B], bf16, tag="ag2o",
                              addr_space="Shared")
            ag3_in = dp.tile([R_HID, B], f32, tag="ag3i")
            ag3_out = dp.tile([NCORES * R_HID, B], f32, tag="ag3o",
                              addr_space="Shared")

            warm_in = dp.tile([NCORES, B], f32, tag="warm_in")
            warm_out = dp.tile([NCORES * NCORES, B], f32, tag="warm_out",
                               addr_space="Shared")
            wz = cpool.tile([NCORES, B], f32, tag="wz")

            with tc.tile_pool(name="psA", bufs=1, space="PSUM") as pA:
                # ---- PE warmup burst + first chunks, emitted ahead of rest
                psu = [pA.tile([128, 512], f32, tag=f"u{j}", name=f"u{j}")
                       for j in range(5)]
                warm_ps = pA.tile([128, 512], f32, tag="warmps")
                warm_z = cpool.tile([128, 512], f32, tag="warmz")
                warm_w = cpool.tile([128, 128], bf16, tag="warmw")
                warm_x = cpool.tile([128, 512], bf16, tag="warmx")
                dumm = cpool.tile([128, 1], f32, tag="dumm")
                mm_marks = {}
                WQ = [nc.sync, nc.scalar]

                def chunk_dma(kc):
                    k0, nk = SUBS[kc]
                    wt = wp.tile([128, CH * H], bf16, tag="wt", name="wt")
                    xt = xp.tile([128, CH * TOK], bf16, tag="xt", name="xt")
                    WQ[kc % 2].dma_start(
                        wt[:, 0:nk * H], d_wT[:, k0 * H:(k0 + nk) * H])
                    WQ[(kc + 1) % 2].dma_start(
                        xt[:, 0:nk * TOK], d_xT[:, k0 * TOK:(k0 + nk) * TOK])
                    return wt, xt

                def chunk_mms(kc, wt, xt):
                    k0, nk = SUBS[kc]
                    for j_in in range(nk):
                        k = k0 + j_in
                        for m in range(MT):
                            j, half = divmod(m, 2)
                            # two 256-wide accumulation groups share each 2KB
                            # PSUM bank: only the even half emits start
                            # (zeroing the whole bank region), only the odd
                            # half emits stop. Accumulation restarts at
                            # k=KSPLIT for the split conv.
                            inst = nc.tensor.matmul(
                                psu[j][:, half * 256:(half + 1) * 256],
                                wt[:, j_in * H + m * 128:
                                   j_in * H + (m + 1) * 128],
                                xt[:, j_in * TOK:(j_in + 1) * TOK],
                                start=(k in (0, KSPLIT) and half == 0),
                                stop=(k in (KSPLIT - 1, KT - 1) and half == 1))
                            if k in (0, KSPLIT - 1, KSPLIT, KT - 1):
                                mm_marks[(k, m)] = inst

                def do_chunk(kc):
                    wt, xt = chunk_dma(kc)
                    chunk_mms(kc, wt, xt)

                # first chunk DMAs go out before anything else; warm-ups
                # run while they are in flight
                c0 = chunk_dma(0)
                c1 = chunk_dma(1)
                c2 = chunk_dma(2)
                nc.vector.memset(warm_z[:], 0.0)
                nc.vector.tensor_copy(warm_w[:], warm_z[:, 0:128])
                nc.vector.tensor_copy(warm_x[:], warm_z[:])
                # preload the one act table (sigmoid_and_others: erf,
                # sigmoid, identity, relu) while the stream runs
                nc.scalar.activation(dumm[:], warm_z[:, 0:1], AF.Erf)
                # tiny AllGather absorbs the ncfw first-collective init
                nc.vector.memset(wz[:], 0.0)
                nc.scalar.dma_start(warm_in[:, :], wz[:])
                nc.gpsimd.collective_compute(
                    "AllGather", OP.bypass, replica_groups=RG,
                    ins=[warm_in.opt()], outs=[warm_out.opt()])
                for _ in range(9):
                    nc.tensor.matmul(warm_ps[:], warm_w[:], warm_x[:],
                                     start=True, stop=True)
                chunk_mms(0, *c0)
                chunk_mms(1, *c1)
                chunk_mms(2, *c2)

                # ---- packed smalls (krev etc. are host-computed); the
                # gpsimd queue is otherwise idle until the tail
                pa = cpool.tile([128, PA_COLS], f32, tag="pa")
                nc.gpsimd.dma_start(pa[:], d_pa)
                psmall = cpool.tile([128, 6], f32, tag="psmall")
                nc.gpsimd.dma_start(psmall[:], d_ps)

                y1 = ev.tile([128, MT * B], f32, tag="y1")
                y2 = ev.tile([128, MT * B], f32, tag="y2")

                def conv_bank(j, y_dst):
                    # fused per PSUM bank (2 m-tiles): y = sum_t u*k_rev
                    # (+ D-skip); bias enters later via the host-computed
                    # bias column of packedA
                    u4 = psu[j][:].rearrange("p (m b t) -> p m b t", m=2,
                                             b=B)
                    kv = pa[:, 2 * j * T:(2 * j + 2) * T].rearrange(
                        "p (m t) -> p m t", m=2).unsqueeze(2).broadcast_to(
                        (128, 2, B, T))
                    pr = ev.tile([128, 512], f32, tag="pr", name="pr")
                    nc.vector.tensor_tensor(
                        pr[:].rearrange("p (m b t) -> p m b t", m=2, b=B),
                        u4, kv, op=OP.mult)
                    y_j = y_dst[:, 2 * j * B:(2 * j + 2) * B]
                    nc.vector.reduce_sum(
                        y_j.rearrange("p (m b) -> p m b", m=2),
                        pr[:].rearrange("p (m b t) -> p m b t", m=2, b=B),
                        axis=mybir.AxisListType.X)
                    dsk = ev.tile([128, 2 * B], f32, tag="dsk", name="dsk")
                    nc.vector.tensor_tensor(
                        dsk[:], u4[:, :, :, T - 1].rearrange(
                            "p m b -> p (m b)"),
                        pa[:, O_D4 + 2 * j * B:O_D4 + (2 * j + 2) * B],
                        op=OP.mult)
                    nc.vector.tensor_add(y_j, y_j, dsk[:])

                def conv_pass(y_dst):
                    for j in range(5):
                        conv_bank(j, y_dst)

                # ---- Phase A: remaining chunks; first conv overlaps stream
                for kc in range(3, len(SUBS) - 1):
                    do_chunk(kc)
                    if SUBS[kc][0] + SUBS[kc][1] == KSPLIT:
                        conv_pass(y1)

                # final sub-chunk in bank-outer order: the tail conv and
                # combine pipeline per-bank behind the PE instead of
                # serializing after the last matmul
                yfb = ev.tile([128, MT * B], bf16, tag="yfb")
                k0f, nkf = SUBS[-1]
                wtf = wp.tile([128, CH * H], bf16, tag="wt", name="wt")
                xtf = xp.tile([128, CH * TOK], bf16, tag="xt", name="xt")
                WQ[(len(SUBS) - 1) % 2].dma_start(
                    wtf[:, 0:nkf * H], d_wT[:, k0f * H:(k0f + nkf) * H])
                WQ[len(SUBS) % 2].dma_start(
                    xtf[:, 0:nkf * TOK], d_xT[:, k0f * TOK:(k0f + nkf) * TOK])
                for j in range(5):
                    for j_in in range(nkf):
                        k = k0f + j_in
                        for half in range(2):
                            m = 2 * j + half
                            inst = nc.tensor.matmul(
                                psu[j][:, half * 256:(half + 1) * 256],
                                wtf[:, j_in * H + m * 128:
                                    j_in * H + (m + 1) * 128],
                                xtf[:, j_in * TOK:(j_in + 1) * TOK],
                                start=False,
                                stop=(k == KT - 1 and half == 1))
                            if k in (0, KSPLIT - 1, KSPLIT, KT - 1):
                                mm_marks[(k, m)] = inst
                    conv_bank(j, y2)
                    sl = slice(2 * j * B, (2 * j + 2) * B)
                    ytmp = ev.tile([128, 2 * B], f32, tag="ytmp",
                                   name="ytmp")
                    nc.vector.tensor_add(ytmp[:], y1[:, sl], y2[:, sl])
                    nc.vector.tensor_tensor(
                        yfb[:, sl], ytmp[:],
                        pa[:, O_BY + 2 * j * B:O_BY + (2 * j + 2) * B],
                        op=OP.add)

                for j in range(5):
                    for kk in (0, KSPLIT - 1, KSPLIT, KT - 1):
                        add_dep_helper(mm_marks[(kk, 2 * j + 1)].ins,
                                       mm_marks[(kk, 2 * j)].ins,
                                       reason="psum zero-region order")

                # epilogue weights: one packed DMA, needed only in phase D
                pb_t = cpool.tile([128, PB_COLS], bf16, tag="pbt")
                nc.gpsimd.dma_start(pb_t[:], d_pb)
                wcs = [pb_t[:, k * 4 * HGO:(k + 1) * 4 * HGO]
                       for k in range(MT)]
                O_W1 = MT * 4 * HGO
                w1a = pb_t[0:HGO, O_W1:O_W1 + R_HID]
                w1b = pb_t[0:HGO, O_W1 + R_HID:O_W1 + 2 * R_HID]
                w2 = pb_t[0:R_HID, O_W1 + 2 * R_HID:O_W1 + 2 * R_HID + NCLS]
                t_b1 = psmall[0:R_HID, 0:1]
                t_b2 = psmall[0:NCLS, 1:2]
                bcg = [psmall[0:HGO, 2 + j:3 + j] for j in range(4)]

                # stage the combined partial for the cross-core exchange
                nc.scalar.dma_start(ag2_in[:, :], yfb[:])

            # AllGather the partial y_last; every core reduces the 8 slots
            nc.gpsimd.collective_compute(
                "AllGather", OP.bypass, replica_groups=RG,
                ins=[ag2_in.opt()], outs=[ag2_out.opt()])

            # ---- Phase D: reduce + GELU + GLU (sharded 160 ch per core)
            with tc.tile_pool(name="de", bufs=1) as de, \
                 tc.tile_pool(name="psB", bufs=1, space="PSUM") as pB:
                yg8 = de.tile([128, NCORES * MT * B], bf16, tag="yg8")
                nc.scalar.dma_start(
                    yg8[:].rearrange("p (s c) -> p s c", s=NCORES),
                    ag2_out.rearrange("(s p) c -> p s c", p=128))
                ysum = de.tile([128, MT * B], f32, tag="ysum")
                nc.vector.reduce_sum(
                    ysum[:],
                    yg8[:].rearrange("p (s c) -> p c s", s=NCORES),
                    axis=mybir.AxisListType.X)
                # gelu(y) = 0.5*y*(1+erf(y/sqrt(2))); the 0.5 is folded
                # into Wc on the host, erf shares the preloaded act table
                erf_t = de.tile([128, MT * B], f32, tag="erft")
                nc.scalar.activation(erf_t[:], ysum[:], AF.Erf,
                                     scale=0.7071067811865476)
                nc.vector.tensor_scalar_add(erf_t[:], erf_t[:], 1.0)
                yg_all = de.tile([128, MT * B], bf16, tag="ygall")
                nc.vector.tensor_mul(yg_all[:], ysum[:], erf_t[:])
                yg = [yg_all[:, m * B:(m + 1) * B] for m in range(MT)]
                z = [pB.tile([HGO, B], f32, tag=f"z{j}", name=f"z{j}")
                     for j in range(4)]
                for k in range(MT):
                    for j in range(4):
                        nc.tensor.matmul(z[j][:],
                                         wcs[k][:, j * HGO:(j + 1) * HGO],
                                         yg[k], start=(k == 0),
                                         stop=(k == MT - 1))
                a0 = de.tile([HGO, B], f32, tag="a0")
                a1 = de.tile([HGO, B], f32, tag="a1")
                s0 = de.tile([HGO, B], f32, tag="s0")
                s1 = de.tile([HGO, B], f32, tag="s1")
                nc.scalar.activation(a0[:], z[0][:], AF.Identity, bias=bcg[0])
                nc.scalar.activation(a1[:], z[1][:], AF.Identity, bias=bcg[1])
                nc.scalar.activation(s0[:], z[2][:], AF.Sigmoid, bias=bcg[2])
                nc.scalar.activation(s1[:], z[3][:], AF.Sigmoid, bias=bcg[3])
                g0 = de.tile([HGO, B], bf16, tag="g0")
                g1 = de.tile([HGO, B], bf16, tag="g1")
                nc.vector.tensor_mul(g0[:], a0[:], s0[:])
                nc.vector.tensor_mul(g1[:], a1[:], s1[:])

                # partial readout: h_p = W1[:, my 160 ch] @ glu_shard
                ps_hp = pB.tile([R_HID, B], f32, tag="php")
                nc.tensor.matmul(ps_hp[:], w1a, g0[:], start=True, stop=False)
                nc.tensor.matmul(ps_hp[:], w1b, g1[:], start=False, stop=True)
                hp_sb = de.tile([R_HID, B], f32, tag="hpsb")
                nc.scalar.activation(hp_sb[:], ps_hp[:], AF.Identity)
                nc.scalar.dma_start(ag3_in[:, :], hp_sb[:])

                nc.gpsimd.collective_compute(
                    "AllGather", OP.bypass, replica_groups=RG,
                    ins=[ag3_in.opt()], outs=[ag3_out.opt()])

                # ---- Phase E: reduce 8 slots, relu, final linear
                hg = de.tile([R_HID, NCORES * B], f32, tag="hg")
                nc.sync.dma_start(
                    hg[:].rearrange("p (s b) -> p s b", s=NCORES),
                    ag3_out.rearrange("(s p) b -> p s b", p=R_HID))
                hsum = de.tile([R_HID, B], f32, tag="hsum")
                nc.vector.reduce_sum(
                    hsum[:],
                    hg[:].rearrange("p (s b) -> p b s", s=NCORES),
                    axis=mybir.AxisListType.X)
                h1 = de.tile([R_HID, B], bf16, tag="h1")
                nc.scalar.activation(h1[:], hsum[:], AF.Relu, bias=t_b1)
                ps_o = pB.tile([NCLS, B], f32, tag="po")
                nc.tensor.matmul(ps_o[:], w2, h1[:], start=True, stop=True)
                o_sb = de.tile([NCLS, B], f32, tag="osb")
                nc.scalar.activation(o_sb[:], ps_o[:], AF.Identity,
                                     bias=t_b2)
                nc.scalar.dma_start(d_out, o_sb[:])

    nc.compile()
    return nc


def _prep_inputs(inputs):
    import ml_dtypes
    x = np.asarray(inputs["x"], dtype=np.float32)
    Wb = np.asarray(inputs["Wb"], dtype=np.float32)
    bb = np.asarray(inputs["bb"], dtype=np.float32)
    log_dt = np.asarray(inputs["log_dt"], dtype=np.float64)
    C = np.asarray(inputs["C"], dtype=np.float64)
    logA = np.asarray(inputs["log_A_real"], dtype=np.float64)
    D = np.asarray(inputs["D"], dtype=np.float32)
    Wc = np.asarray(inputs["Wc"], dtype=np.float32)
    bc = np.asarray(inputs["bc"], dtype=np.float32)
    W1 = np.asarray(inputs["W1"], dtype=np.float32)
    b1 = np.asarray(inputs["b1"], dtype=np.float32)
    W2 = np.asarray(inputs["W2"], dtype=np.float32)
    b2 = np.asarray(inputs["b2"], dtype=np.float32)

    xT = np.ascontiguousarray(x.reshape(TOK, DIN).T).astype(ml_dtypes.bfloat16)
    wT = np.ascontiguousarray(Wb.T).astype(ml_dtypes.bfloat16)
    WcT = np.ascontiguousarray(Wc.T)                     # (1280, 2560)
    W1T = np.ascontiguousarray(W1.T)                     # (1280, 64)
    W2T = np.ascontiguousarray(W2.T)                     # (64, 60)

    # host-side S4D kernel: krev[h, t] = 2*sum_n C*(exp(dtA)-1)/A
    #                                      * exp(dtA*(T-1-t))
    dt = np.exp(log_dt)                                  # (H,)
    A = -np.exp(logA)                                    # (H, N2)
    dtA = A * dt[:, None]
    cb2 = 2.0 * C * (np.exp(dtA) - 1.0) / A              # (H, N2)
    rev = np.arange(T - 1, -1, -1, dtype=np.float64)
    krev = np.einsum("hn,hnt->ht", cb2,
                     np.exp(dtA[:, :, None] * rev)).astype(np.float32)
    # bias path: u's +bb contributes bb*(sum_t krev + D) to y_last;
    # divided by NCORES because the 8 partial slots are summed
    bias_y = (bb * (krev.sum(1) + D) / NCORES).astype(np.float32)  # (H,)

    # partition-major repack: arr_p[p, k, :] = arr[k*128+p, :]
    pm = lambda a: np.ascontiguousarray(
        a.reshape(-1, 128, a.shape[-1]).transpose(1, 0, 2)).reshape(128, -1)

    packedA = np.ascontiguousarray(np.concatenate(
        [pm(krev),
         np.repeat(pm(D.reshape(H, 1)), B, axis=1),
         np.repeat(pm(bias_y.reshape(H, 1)), B, axis=1)], axis=1))

    def pad128(a):
        out = np.zeros((128, a.shape[1]), np.float32)
        out[:a.shape[0]] = a
        return out

    in_maps = []
    for i in range(NCORES):
        klo = i * KS
        go = i * GO
        wTp = np.ascontiguousarray(
            wT[klo:klo + KS].reshape(KT, 128, H).transpose(1, 0, 2)
        ).reshape(128, KT * H)
        xTp = np.ascontiguousarray(
            xT[klo:klo + KS].reshape(KT, 128, TOK).transpose(1, 0, 2)
        ).reshape(128, KT * TOK)
        wcT_sl = np.concatenate(
            [WcT[:, go:go + HGO], WcT[:, go + HGO:go + GO],
             WcT[:, H + go:H + go + HGO], WcT[:, H + go + HGO:H + go + GO]],
            axis=1) * 0.5    # 0.5 of the erf-based gelu  # (1280, 320)
        packedB = np.ascontiguousarray(np.concatenate(
            [pm(wcT_sl), pad128(W1T[go:go + HGO]),
             pad128(W1T[go + HGO:go + GO]), pad128(W2T)],
            axis=1)).astype(ml_dtypes.bfloat16)
        bc_sl = np.stack(
            [bc[go:go + HGO], bc[go + HGO:go + GO],
             bc[H + go:H + go + HGO], bc[H + go + HGO:H + go + GO]],
            axis=1)                                     # (80, 4)
        packedS = np.ascontiguousarray(np.concatenate(
            [pad128(b1.reshape(R_HID, 1)), pad128(b2.reshape(NCLS, 1)),
             pad128(bc_sl)], axis=1))
        in_maps.append({
            "xT": xTp, "wT": wTp, "packedA": packedA,
            "packedB": packedB, "packedS": packedS,
        })
    return in_maps


def kernel(**inputs):
    global _compiled
    if _compiled is None:
        _compiled = _build()
    nc = _compiled
    in_maps = _prep_inputs(inputs)
    from concourse import bass_utils
    res = bass_utils.run_bass_kernel_spmd(nc, in_maps,
                                          core_ids=list(range(NCORES)))
    out = res.results[0]["out"]  # (NCLS, B)
    return np.ascontiguousarray(out.T).astype(np.float32)


# revision 46
# speedup vs baseline: 1.2930x; 1.2930x over previous
"""Trainium2 Bass kernel for nn_BottleneckS4D (8-core SPMD).

Strategy (self-contained, hardcoded):
  The reference is  u = x_flat @ Wb.T + bb  (256 x 150528 @ 150528 x 1280,
  770MB weight) followed by an S4D block whose output is only consumed at
  the LAST timestep (readout takes y[:, -1, :]), so the FFT convolution
  collapses to a per-channel dot product over time with the reversed S4D
  kernel, and everything downstream is tiny.

  Sharding: split the CONTRACTION dim D_IN=150528 across the 8 cores
  (18816 each). Each core streams its weight slice + x slice once in
  bf16 (48.2MB + 9.6MB, the HBM-traffic minimum at passing precision;
  fp8 fails the 2e-2 gate at ~2.5-3.6e-2) and computes a partial u^T
  (1280, 256) in PSUM with bf16 matmuls at full PE rate. The S4D conv
  is linear in u, so each core reduces its partial u to a partial
  y_last (1280, 4) with the host-precomputed reversed kernel k_rev;
  the cross-core sum uses AllGather + on-core reduction (a mesh
  AllReduce costs ~27us here, AllGather only ~7us). GELU + the GLU
  1x1 conv run on a 160-channel shard per core, a partial readout
  matvec reduces the shard to (64, 4), and a second tiny AllGather +
  reduction gives every core the readout input; core 0's output is
  returned.

  Perf details: weights/x are host-repacked to partition-major layout
  (wTp[p, k, :] = wT[k*128+p, :]) so each DMA chunk moves its k-tiles
  with one large contiguous descriptor per partition; w/x chunks
  alternate between the sync and scalar HWDGE queues (gpsimd carries
  only late-needed smalls — its queue blocks behind collective
  triggers); krev, the D-skip vector and the bias-path term
  bb*(sum_t krev + D)/8 are computed on the host (they depend only on
  weights) so the device does no S4D prep; small first chunks and a PE
  warmup burst start the stream fast while the HAM clock gate lifts;
  GELU is computed as 0.5*y*(1+erf(y/sqrt2)) with the 0.5 folded into
  Wc so every tail activation (erf/sigmoid/identity/relu) lives in one
  act-function set, preloaded during the stream; PSUM accumulation
  restarts at k=KSPLIT so the conv of the first half overlaps the
  remaining stream, and the final sub-chunk runs bank-outer so the tail
  conv pipelines behind the PE; the epilogue (GLU + readout) weights
  are bf16; a tiny AllGather early in the kernel absorbs the ncfw
  first-collective init concurrently with the stream.
"""
import sys

sys.path.insert(0, "/opt/trn_rl_repo")
import numpy as np

B, T, H, N2 = 4, 64, 1280, 32
DIN = 224 * 224 * 3  # 150528
R_HID, NCLS = 64, 60
NCORES = 8
KS = DIN // NCORES   # 18816
KT = KS // 128       # 147
MT = H // 128        # 10
TOK = B * T          # 256
GO = H // NCORES     # 160 GLU output channels per core
HGO = GO // 2        # 80
CH = 7               # max k-tiles per DMA chunk (SBUF slot size)
# sub-chunk schedule: small chunks first so the PE starts fast, then
# 7-k-tile chunks; alternates between the sync and scalar HWDGE queues
SUBS = [(0, 2), (2, 2), (4, 3), (7, 4), (11, 3)] + \
       [(14 + 7 * i, 7) for i in range(19)]
KSPLIT = 98          # conv of k<KSPLIT overlaps the remaining stream
# packedA cols: krev (MT*T) | D4 (MT*B, b-replicated) | biasy (MT*B)
PA_COLS = MT * T + 2 * MT * B  # 720
# packedB cols: wcT(10*320) | w1a(64) | w1b(64) | w2(60)
PB_COLS = MT * 4 * HGO + 2 * R_HID + NCLS  # 3388

_compiled = None


def _build():
    import concourse.bacc as bacc
    import concourse.mybir as mybir
    import concourse.tile as tile
    from concourse.tile import add_dep_helper

    f32 = mybir.dt.float32
    f32r = mybir.dt.float32r
    bf16 = mybir.dt.bfloat16
    AF = mybir.ActivationFunctionType
    OP = mybir.AluOpType
    RG = [list(range(NCORES))]

    nc = bacc.Bacc("TRN2", target_bir_lowering=False, debug=False,
                   num_devices=NCORES)

    d_xT = nc.dram_tensor("xT", [128, KT * TOK], bf16, kind="ExternalInput").ap()
    d_wT = nc.dram_tensor("wT", [128, KT * H], bf16, kind="ExternalInput").ap()
    d_pa = nc.dram_tensor("packedA", [128, PA_COLS], f32,
                          kind="ExternalInput").ap()
    d_pb = nc.dram_tensor("packedB", [128, PB_COLS], bf16,
                          kind="ExternalInput").ap()
    # packedS cols: b1 | b2 | bc(4 cols, 80 rows)
    d_ps = nc.dram_tensor("packedS", [128, 6], f32, kind="ExternalInput").ap()
    d_out = nc.dram_tensor("out", [NCLS, B], f32, kind="ExternalOutput").ap()

    O_D4 = MT * T        # 640
    O_BY = MT * T + MT * B  # 680

    with tile.TileContext(nc) as tc:
        with tc.tile_pool(name="cpool", bufs=1) as cpool, \
             tc.tile_pool(name="dram", bufs=1, space="DRAM") as dp, \
             tc.tile_pool(name="wp", bufs=6) as wp, \
             tc.tile_pool(name="xp", bufs=6) as xp, \
             tc.tile_pool(name="ev", bufs=3) as ev:
            # cross-core exchange buffers: raw [128, MT*B] layout so every
            # scatter/gather DMA keeps >=80B contiguous runs per partition
            ag2_in = dp.tile([128, MT * B], bf16, tag="ag2i")
            ag2_out = dp.tile([NCORES * 128, MT * B], bf16, tag="ag2o",
                              addr_space="Shared")
            ag3_in = dp.tile([R_HID, B], f32, tag="ag3i")
            ag3_out = dp.tile([NCORES * R_HID, B], f32, tag="ag3o",
                              addr_space="Shared")

            # ---- collective warmup: tiny AllGather absorbs the ncfw
            # first-collective init concurrently with the matmul stream
            warm_in = dp.tile([NCORES, B], f32, tag="warm_in")
            warm_out = dp.tile([NCORES * NCORES, B], f32, tag="warm_out",
                               addr_space="Shared")
            wz = cpool.tile([NCORES, B], f32, tag="wz")
            nc.vector.memset(wz[:], 0.0)
            nc.scalar.dma_start(warm_in[:, :], wz[:])
            nc.gpsimd.collective_compute(
                "AllGather", OP.bypass, replica_groups=RG,
                ins=[warm_in.opt()], outs=[warm_out.opt()])

            with tc.tile_pool(name="psA", bufs=1, space="PSUM") as pA:
                # ---- PE warmup burst + first chunks, emitted ahead of rest
                psu = [pA.tile([128, 512], f32, tag=f"u{j}", name=f"u{j}")
                       for j in range(5)]
                warm_ps = pA.tile([128, 512], f32, tag="warmps")
                warm_z = cpool.tile([128, 512], f32, tag="warmz")
                warm_w = cpool.tile([128, 128], bf16, tag="warmw")
                warm_x = cpool.tile([128, 512], bf16, tag="warmx")
                dumm = cpool.tile([128, 1], f32, tag="dumm")
                nc.vector.memset(warm_z[:], 0.0)
                nc.vector.tensor_copy(warm_w[:], warm_z[:, 0:128])
                nc.vector.tensor_copy(warm_x[:], warm_z[:])
                # preload the one act table (sigmoid_and_others: erf,
                # sigmoid, identity, relu) while the stream runs
                nc.scalar.activation(dumm[:], warm_z[:, 0:1], AF.Erf)
                for _ in range(12):
                    nc.tensor.matmul(warm_ps[:], warm_w[:], warm_x[:],
                                     start=True, stop=True)

                mm_marks = {}
                WQ = [nc.sync, nc.scalar]

                def do_chunk(kc):
                    k0, nk = SUBS[kc]
                    wt = wp.tile([128, CH * H], bf16, tag="wt", name="wt")
                    xt = xp.tile([128, CH * TOK], bf16, tag="xt", name="xt")
                    WQ[kc % 2].dma_start(
                        wt[:, 0:nk * H], d_wT[:, k0 * H:(k0 + nk) * H])
                    WQ[(kc + 1) % 2].dma_start(
                        xt[:, 0:nk * TOK], d_xT[:, k0 * TOK:(k0 + nk) * TOK])
                    for j_in in range(nk):
                        k = k0 + j_in
                        for m in range(MT):
                            j, half = divmod(m, 2)
                            # two 256-wide accumulation groups share each 2KB
                            # PSUM bank: only the even half emits start
                            # (zeroing the whole bank region), only the odd
                            # half emits stop. Accumulation restarts at
                            # k=KSPLIT for the split conv.
                            inst = nc.tensor.matmul(
                                psu[j][:, half * 256:(half + 1) * 256],
                                wt[:, j_in * H + m * 128:
                                   j_in * H + (m + 1) * 128],
                                xt[:, j_in * TOK:(j_in + 1) * TOK],
                                start=(k in (0, KSPLIT) and half == 0),
                                stop=(k in (KSPLIT - 1, KT - 1) and half == 1))
                            if k in (0, KSPLIT - 1, KSPLIT, KT - 1):
                                mm_marks[(k, m)] = inst

                do_chunk(0)
                do_chunk(1)

                # ---- packed smalls (krev etc. are host-computed); the
                # gpsimd queue is otherwise idle until the tail
                pa = cpool.tile([128, PA_COLS], f32, tag="pa")
                nc.gpsimd.dma_start(pa[:], d_pa)
                psmall = cpool.tile([128, 6], f32, tag="psmall")
                nc.gpsimd.dma_start(psmall[:], d_ps)

                y1 = ev.tile([128, MT * B], f32, tag="y1")
                y2 = ev.tile([128, MT * B], f32, tag="y2")

                def conv_bank(j, y_dst):
                    # fused per PSUM bank (2 m-tiles): y = sum_t u*k_rev
                    # (+ D-skip); bias enters later via the host-computed
                    # bias column of packedA
                    u4 = psu[j][:].rearrange("p (m b t) -> p m b t", m=2,
                                             b=B)
                    kv = pa[:, 2 * j * T:(2 * j + 2) * T].rearrange(
                        "p (m t) -> p m t", m=2).unsqueeze(2).broadcast_to(
                        (128, 2, B, T))
                    pr = ev.tile([128, 512], f32, tag="pr", name="pr")
                    nc.vector.tensor_tensor(
                        pr[:].rearrange("p (m b t) -> p m b t", m=2, b=B),
                        u4, kv, op=OP.mult)
                    y_j = y_dst[:, 2 * j * B:(2 * j + 2) * B]
                    nc.vector.reduce_sum(
                        y_j.rearrange("p (m b) -> p m b", m=2),
                        pr[:].rearrange("p (m b t) -> p m b t", m=2, b=B),
                        axis=mybir.AxisListType.X)
                    dsk = ev.tile([128, 2 * B], f32, tag="dsk", name="dsk")
                    nc.vector.tensor_tensor(
                        dsk[:], u4[:, :, :, T - 1].rearrange(
                            "p m b -> p (m b)"),
                        pa[:, O_D4 + 2 * j * B:O_D4 + (2 * j + 2) * B],
                        op=OP.mult)
                    nc.vector.tensor_add(y_j, y_j, dsk[:])

                def conv_pass(y_dst):
                    for j in range(5):
                        conv_bank(j, y_dst)

                # ---- Phase A: remaining chunks; first conv overlaps stream
                for kc in range(2, len(SUBS) - 1):
                    do_chunk(kc)
                    if SUBS[kc][0] + SUBS[kc][1] == KSPLIT:
                        conv_pass(y1)

                # final sub-chunk in bank-outer order: the tail conv and
                # combine pipeline per-bank behind the PE instead of
                # serializing after the last matmul
                yfb = ev.tile([128, MT * B], bf16, tag="yfb")
                k0f, nkf = SUBS[-1]
                wtf = wp.tile([128, CH * H], bf16, tag="wt", name="wt")
                xtf = xp.tile([128, CH * TOK], bf16, tag="xt", name="xt")
                WQ[(len(SUBS) - 1) % 2].dma_start(
                    wtf[:, 0:nkf * H], d_wT[:, k0f * H:(k0f + nkf) * H])
                WQ[len(SUBS) % 2].dma_start(
                    xtf[:, 0:nkf * TOK], d_xT[:, k0f * TOK:(k0f + nkf) * TOK])
                for j in range(5):
                    for j_in in range(nkf):
                        k = k0f + j_in
                        for half in range(2):
                            m = 2 * j + half
                            inst = nc.tensor.matmul(
                                psu[j][:, half * 256:(half + 1) * 256],
                                wtf[:, j_in * H + m * 128:
                                    j_in * H + (m + 1) * 128],
                                xtf[:, j_in * TOK:(j_in + 1) * TOK],
                                start=False,
                                stop=(k == KT - 1 and half == 1))
                            if k in (0, KSPLIT - 1, KSPLIT, KT - 1):
                                mm_marks[(k, m)] = inst
                    conv_bank(j, y2)
                    sl = slice(2 * j * B, (2 * j + 2) * B)
                    ytmp = ev.tile([128, 2 * B], f32, tag="ytmp",
                                   name="ytmp")
                    nc.vector.tensor_add(ytmp[:], y1[:, sl], y2[:, sl])
                    nc.vector.tensor_tensor(
                        yfb[:, sl], ytmp[:],
                        pa[:, O_BY + 2 * j * B:O_BY + (2 * j + 2) * B],
                        op=OP.add)

                # dummy burst holds the core's activity (and HAM clock,
                # which otherwise drops to half) through the AG#2 window;
                # sized to finish before the gathered data arrives so it
                # cannot delay real work
                for _ in range(32):
                    nc.tensor.matmul(warm_ps[:], warm_w[:], warm_x[:],
                                     start=True, stop=True)

                for j in range(5):
                    for kk in (0, KSPLIT - 1, KSPLIT, KT - 1):
                        add_dep_helper(mm_marks[(kk, 2 * j + 1)].ins,
                                       mm_marks[(kk, 2 * j)].ins,
                                       reason="psum zero-region order")

                # epilogue weights: one packed DMA, needed only in phase D
                pb_t = cpool.tile([128, PB_COLS], bf16, tag="pbt")
                nc.gpsimd.dma_start(pb_t[:], d_pb)
                wcs = [pb_t[:, k * 4 * HGO:(k + 1) * 4 * HGO]
                       for k in range(MT)]
                O_W1 = MT * 4 * HGO
                w1a = pb_t[0:HGO, O_W1:O_W1 + R_HID]
                w1b = pb_t[0:HGO, O_W1 + R_HID:O_W1 + 2 * R_HID]
                w2 = pb_t[0:R_HID, O_W1 + 2 * R_HID:O_W1 + 2 * R_HID + NCLS]
                t_b1 = psmall[0:R_HID, 0:1]
                t_b2 = psmall[0:NCLS, 1:2]
                bcg = [psmall[0:HGO, 2 + j:3 + j] for j in range(4)]

                # stage the combined partial for the cross-core exchange
                nc.scalar.dma_start(ag2_in[:, :], yfb[:])

            # AllGather the partial y_last; every core reduces the 8 slots
            nc.gpsimd.collective_compute(
                "AllGather", OP.bypass, replica_groups=RG,
                ins=[ag2_in.opt()], outs=[ag2_out.opt()])

            # ---- Phase D: reduce + GELU + GLU (sharded 160 ch per core)
            with tc.tile_pool(name="de", bufs=1) as de, \
                 tc.tile_pool(name="psB", bufs=1, space="PSUM") as pB:
                yg8 = de.tile([128, NCORES * MT * B], bf16, tag="yg8")
                nc.scalar.dma_start(
                    yg8[:].rearrange("p (s c) -> p s c", s=NCORES),
                    ag2_out.rearrange("(s p) c -> p s c", p=128))
                ysum = de.tile([128, MT * B], f32, tag="ysum")
                nc.vector.reduce_sum(
                    ysum[:],
                    yg8[:].rearrange("p (s c) -> p c s", s=NCORES),
                    axis=mybir.AxisListType.X)
                # gelu(y) = 0.5*y*(1+erf(y/sqrt(2))); the 0.5 is folded
                # into Wc on the host, erf shares the preloaded act table
                erf_t = de.tile([128, MT * B], f32, tag="erft")
                nc.scalar.activation(erf_t[:], ysum[:], AF.Erf,
                                     scale=0.7071067811865476)
                nc.vector.tensor_scalar_add(erf_t[:], erf_t[:], 1.0)
                yg_all = de.tile([128, MT * B], bf16, tag="ygall")
                nc.vector.tensor_mul(yg_all[:], ysum[:], erf_t[:])
                yg = [yg_all[:, m * B:(m + 1) * B] for m in range(MT)]
                z = [pB.tile([HGO, B], f32, tag=f"z{j}", name=f"z{j}")
                     for j in range(4)]
                for k in range(MT):
                    for j in range(4):
                        nc.tensor.matmul(z[j][:],
                                         wcs[k][:, j * HGO:(j + 1) * HGO],
                                         yg[k], start=(k == 0),
                                         stop=(k == MT - 1))
                a0 = de.tile([HGO, B], f32, tag="a0")
                a1 = de.tile([HGO, B], f32, tag="a1")
                s0 = de.tile([HGO, B], f32, tag="s0")
                s1 = de.tile([HGO, B], f32, tag="s1")
                nc.scalar.activation(a0[:], z[0][:], AF.Identity, bias=bcg[0])
                nc.scalar.activation(a1[:], z[1][:], AF.Identity, bias=bcg[1])
                nc.scalar.activation(s0[:], z[2][:], AF.Sigmoid, bias=bcg[2])
                nc.scalar.activation(s1[:], z[3][:], AF.Sigmoid, bias=bcg[3])
                g0 = de.tile([HGO, B], bf16, tag="g0")
                g1 = de.tile([HGO, B], bf16, tag="g1")
                nc.vector.tensor_mul(g0[:], a0[:], s0[:])
                nc.vector.tensor_mul(g1[:], a1[:], s1[:])

                # partial readout: h_p = W1[:, my 160 ch] @ glu_shard
                ps_hp = pB.tile([R_HID, B], f32, tag="php")
                nc.tensor.matmul(ps_hp[:], w1a, g0[:], start=True, stop=False)
                nc.tensor.matmul(ps_hp[:], w1b, g1[:], start=False, stop=True)
                hp_sb = de.tile([R_HID, B], f32, tag="hpsb")
                nc.scalar.activation(hp_sb[:], ps_hp[:], AF.Identity)
                nc.scalar.dma_start(ag3_in[:, :], hp_sb[:])

                nc.gpsimd.collective_compute(
                    "AllGather", OP.bypass, replica_groups=RG,
                    ins=[ag3_in.opt()], outs=[ag3_out.opt()])

                # ---- Phase E: reduce 8 slots, relu, final linear
                hg = de.tile([R_HID, NCORES * B], f32, tag="hg")
                nc.sync.dma_start(
                    hg[:].rearrange("p (s b) -> p s b", s=NCORES),
                    ag3_out.rearrange("(s p) b -> p s b", p=R_HID))
                hsum = de.tile([R_HID, B], f32, tag="hsum")
                nc.vector.reduce_sum(
                    hsum[:],
                    hg[:].rearrange("p (s b) -> p b s", s=NCORES),
                    axis=mybir.AxisListType.X)
                h1 = de.tile([R_HID, B], bf16, tag="h1")
                nc.scalar.activation(h1[:], hsum[:], AF.Relu, bias=t_b1)
                ps_o = pB.tile([NCLS, B], f32, tag="po")
                nc.tensor.matmul(ps_o[:], w2, h1[:], start=True, stop=True)
                o_sb = de.tile([NCLS, B], f32, tag="osb")
                nc.scalar.activation(o_sb[:], ps_o[:], AF.Identity,
                                     bias=t_b2)
                nc.scalar.dma_start(d_out, o_sb[:])

    nc.compile()
    return nc


def _prep_inputs(inputs):
    import ml_dtypes
    x = np.asarray(inputs["x"], dtype=np.float32)
    Wb = np.asarray(inputs["Wb"], dtype=np.float32)
    bb = np.asarray(inputs["bb"], dtype=np.float32)
    log_dt = np.asarray(inputs["log_dt"], dtype=np.float64)
    C = np.asarray(inputs["C"], dtype=np.float64)
    logA = np.asarray(inputs["log_A_real"], dtype=np.float64)
    D = np.asarray(inputs["D"], dtype=np.float32)
    Wc = np.asarray(inputs["Wc"], dtype=np.float32)
    bc = np.asarray(inputs["bc"], dtype=np.float32)
    W1 = np.asarray(inputs["W1"], dtype=np.float32)
    b1 = np.asarray(inputs["b1"], dtype=np.float32)
    W2 = np.asarray(inputs["W2"], dtype=np.float32)
    b2 = np.asarray(inputs["b2"], dtype=np.float32)

    xT = np.ascontiguousarray(x.reshape(TOK, DIN).T).astype(ml_dtypes.bfloat16)
    wT = np.ascontiguousarray(Wb.T).astype(ml_dtypes.bfloat16)
    WcT = np.ascontiguousarray(Wc.T)                     # (1280, 2560)
    W1T = np.ascontiguousarray(W1.T)                     # (1280, 64)
    W2T = np.ascontiguousarray(W2.T)                     # (64, 60)

    # host-side S4D kernel: krev[h, t] = 2*sum_n C*(exp(dtA)-1)/A
    #                                      * exp(dtA*(T-1-t))
    dt = np.exp(log_dt)                                  # (H,)
    A = -np.exp(logA)                                    # (H, N2)
    dtA = A * dt[:, None]
    cb2 = 2.0 * C * (np.exp(dtA) - 1.0) / A              # (H, N2)
    rev = np.arange(T - 1, -1, -1, dtype=np.float64)
    krev = np.einsum("hn,hnt->ht", cb2,
                     np.exp(dtA[:, :, None] * rev)).astype(np.float32)
    # bias path: u's +bb contributes bb*(sum_t krev + D) to y_last;
    # divided by NCORES because the 8 partial slots are summed
    bias_y = (bb * (krev.sum(1) + D) / NCORES).astype(np.float32)  # (H,)

    # partition-major repack: arr_p[p, k, :] = arr[k*128+p, :]
    pm = lambda a: np.ascontiguousarray(
        a.reshape(-1, 128, a.shape[-1]).transpose(1, 0, 2)).reshape(128, -1)

    packedA = np.ascontiguousarray(np.concatenate(
        [pm(krev),
         np.repeat(pm(D.reshape(H, 1)), B, axis=1),
         np.repeat(pm(bias_y.reshape(H, 1)), B, axis=1)], axis=1))

    def pad128(a):
        out = np.zeros((128, a.shape[1]), np.float32)
        out[:a.shape[0]] = a
        return out

    in_maps = []
    for i in range(NCORES):
        klo = i * KS
        go = i * GO
        wTp = np.ascontiguousarray(
            wT[klo:klo + KS].reshape(KT, 128, H).transpose(1, 0, 2)
        ).reshape(128, KT * H)
        xTp = np.ascontiguousarray(
            xT[klo:klo + KS].reshape(KT, 128, TOK).transpose(1, 0, 2)
        ).reshape(128, KT * TOK)
        wcT_sl = np.concatenate(
            [WcT[:, go:go + HGO], WcT[:, go + HGO:go + GO],
             WcT[:, H + go:H + go + HGO], WcT[:, H + go + HGO:H + go + GO]],
            axis=1) * 0.5    # 0.5 of the erf-based gelu  # (1280, 320)
        packedB = np.ascontiguousarray(np.concatenate(
            [pm(wcT_sl), pad128(W1T[go:go + HGO]),
             pad128(W1T[go + HGO:go + GO]), pad128(W2T)],
            axis=1)).astype(ml_dtypes.bfloat16)
        bc_sl = np.stack(
            [bc[go:go + HGO], bc[go + HGO:go + GO],
             bc[H + go:H + go + HGO], bc[H + go + HGO:H + go + GO]],
            axis=1)                                     # (80, 4)
        packedS = np.ascontiguousarray(np.concatenate(
            [pad128(b1.reshape(R_HID, 1)), pad128(b2.reshape(NCLS, 1)),
             pad128(bc_sl)], axis=1))
        in_maps.append({
            "xT": xTp, "wT": wTp, "packedA": packedA,
            "packedB": packedB, "packedS": packedS,
        })
    return in_maps


def kernel(**inputs):
    global _compiled
    if _compiled is None:
        _compiled = _build()
    nc = _compiled
    in_maps = _prep_inputs(inputs)
    from concourse import bass_utils
    res = bass_utils.run_bass_kernel_spmd(nc, in_maps,
                                          core_ids=list(range(NCORES)))
    out = res.results[0]["out"]  # (NCLS, B)
    return np.ascontiguousarray(out.T).astype(np.float32)
